# revision 29
# baseline (speedup 1.0000x reference)
"""Trainium2 Bass kernel for nn_NeuronS3DiffUpsample2D.

Reference computation (per sample b):
    up   = nearest-2x-upsample(x[b])                       # [C, 320, 320]
    w    = Wb + 0.25 * einsum('or,rikl->oikl', lora_up, lora_down)
    w_b  = w * de_mod[b, None, :, None, None]              # modulate input chans
    dem  = rsqrt(sum_{i,k,l} w_b^2 + eps)                  # per output chan
    y[b] = conv2d(up, w_b * dem, SAME) + bias

Key algebraic transform: a 3x3 SAME conv on a 2x nearest-upsampled image
decomposes into 4 output phases (di, dj in {0,1}), each a 2x2 conv on the
ORIGINAL 160x160 input:
    y[2i+di, 2j+dj] = sum_{a,b in {0,1}} K[di,dj,a,b] @ x[i+a+di-1, j+b+dj-1]
where each K[di,dj,a,b] is a row-combo x col-combo sum of the 9 taps of w:
  row-combos (di,a): {w0, w1+w2, w0+w1, w2} over ki; same pattern over kj.
This is 4/9 of the naive FLOPs and never materializes the upsampled image.

Since the demod scale is per output channel and conv is linear in w, the conv
OUTPUT is scaled by dem[o] (per-partition scalar) at PSUM eviction, fused with
the bias add; weights are only modulated by de_mod on the input-channel axis.

Sharding: data-parallel over batch B=8 across 8 NeuronCores; each core builds
its own per-sample weights locally (replicated W/lora are tiny).

Performance notes (from perfetto traces of earlier revisions):
  * The conv loop is a zero-gap matmul stream; its cadence was set by f32r
    LDWEIGHTS (224 ns > the 200 ns N=480 matmul).  All matmul operands are
    bf16 now: LDWEIGHTS takes ~107 ns (with FWL) and hides fully, and the
    input DMA bytes halve.  Accumulation stays fp32 in PSUM; rel err ~2e-3
    against the fp32 reference.
  * x is padded to [C,162,162] with a zero border ON HOST so every band DMA
    is a single contiguous descriptor per partition (no SWDGE descriptor
    storms, no DVE border memsets) and arrives fast.
  * Of the 16 combined-tap matrices, 8 are direct views into the row-combo
    tiles (no copies); only the 8 column-sums are materialized by DVE.
  * The demod reduction uses 4 contiguous DVE adds instead of one strided
    tensor_reduce; its tiny PE matmul is scheduled before the conv stream so
    the PSUM pool for the conv loop can own all 8 banks.
"""

import sys
import numpy as np
from contextlib import ExitStack

try:
    import concourse.bass as bass
except ImportError:  # grading env without the axon PYTHONPATH
    sys.path.insert(0, "/opt/trn_rl_repo")
    import concourse.bass as bass
import concourse.tile as tile
from concourse import bacc, mybir
from concourse.bass_utils import run_bass_kernel_spmd

B, C, H, W = 8, 128, 160, 160
RANK = 32
SCALING = 0.25
EPS = 1e-8
HP, WP = H + 2, W + 2   # zero-padded image (1-px border baked in on host)
R_BLK = 3               # x-rows per matmul block -> N = 3*160 = 480 <= 512
C9 = 9 * C
NCORES = 8

# Input bands (padded-row ranges).  Block i0 needs padded rows [i0, i0+4];
# bands overlap by 4 rows so any block reads from a single tile.  The first
# band is small so the conv stream can start as soon as the weight stage is
# done; later bands are large to amortize DMA setup.
BANDS = [(0, 14), (12, 26), (24, 50), (48, 86), (84, 124), (120, 162)]

f32 = mybir.dt.float32
bf16 = mybir.dt.bfloat16


def _band_of(i0):
    if i0 <= 9:
        return 0
    if i0 <= 21:
        return 1
    if i0 <= 45:
        return 2
    if i0 <= 81:
        return 3
    if i0 <= 117:
        return 4
    return 5


def _conv_kernel(ctx, tc, y, x, wpk):
    nc = tc.nc
    AF = mybir.ActivationFunctionType
    ALU = mybir.AluOpType

    const = ctx.enter_context(tc.tile_pool(name="const", bufs=1))

    demP = const.tile([128, 1], f32)         # rsqrt demod, per output chan
    evb = const.tile([128, 1], f32)          # bias[o], f32 for evictions
    dmf = const.tile([128, 1], f32)          # de_mod[i], f32 scalar operand
    wm3 = const.tile([128, C9], bf16)        # modulated 9-tap weights [i,(t o)]
    R01 = const.tile([128, 3 * C], bf16)     # row-combo ki1+ki2
    R10 = const.tile([128, 3 * C], bf16)     # row-combo ki0+ki1
    cmb = const.tile([128, 4, 2, C], bf16)   # col-sums per (di,a): [A=kj1+kj2, B=kj0+kj1]
    # Wb^T [i,(t o)] + de_mod col + bias col + (rows 0-31) lora pack.  One
    # tensor -> ONE weight DMA: a second DMA on the same HWDGE ring pays a
    # ~4us completion penalty that would gate the whole weight stage.
    W9 = const.tile([128, C9 + 2 + 10 * C], bf16)

    # x bands: contiguous 1-descriptor-per-partition DMAs on the otherwise
    # idle GpSimd queue (separate from the weight DMAs on sync and the
    # output DMAs on sync).  band0 is issued immediately; bands 1-4 are
    # held behind a probe op that depends on the W9 weight DMA so their
    # bulk transfers don't steal SDMA engines from the weight stage.
    band_tiles = []
    for bi, (s, e) in enumerate(BANDS):
        bt = const.tile([128, e - s, WP], bf16, name=f"band{bi}")
        band_tiles.append((bt, s))

    dmv = W9[:, C9 : C9 + 1]                 # de_mod[i] per partition
    biasv = W9[:, C9 + 1 : C9 + 2]

    wtmp = ctx.enter_context(tc.tile_pool(name="wtmp", bufs=1))
    spsum = ctx.enter_context(tc.tile_pool(name="spsum", bufs=1, space="PSUM"))
    with tc.tile_pool(name="wpsum", bufs=1, space="PSUM") as wpsum:
        nc.sync.dma_start(W9[:], wpk[:])
        LOR = W9[0:RANK, C9 + 2 : C9 + 2 + 10 * C]   # [lora_down^T | 0.25*lora_up^T]

        # Bands are chained: band k+1's descriptor generation waits (via a
        # 1-element probe) for band k's completion.  This pins the transfer
        # order band0 < band1 < ... (the scheduler otherwise reorders the
        # descgens and a late band stalls the conv stream) and keeps early
        # band traffic from flooding the SDMA engines all at once.
        for bi in range(len(BANDS)):
            bt, s = band_tiles[bi]
            nc.gpsimd.dma_start(bt[:], x[:, s : BANDS[bi][1], :])
            if bi + 1 < len(BANDS):
                bprobe = wtmp.tile([1, 1], bf16, name=f"bprobe{bi}")
                nc.gpsimd.tensor_copy(bprobe[:], bt[0:1, 0, 0:1])
        wsum = wtmp.tile([128, C9], bf16)

        # DMA-independent PE warm-up fuel: the first warm matmuls must not
        # wait for any DMA, so the HAM activity window starts filling at
        # ~7us and the clock gate is open before the conv stream begins.
        warm0 = wtmp.tile([128, 480], bf16)
        nc.vector.memset(warm0[:], 0.25)
        epsA = wtmp.tile([128, 1], f32)
        nc.vector.memset(epsA[:], EPS)
        nc.vector.tensor_copy(evb[:], biasv)
        nc.vector.tensor_copy(dmf[:], dmv)
        dm2 = wtmp.tile([128, 1], bf16)          # de_mod[i]^2, demod matmul rhs
        nc.scalar.square(dm2[:], dmf[:])

        # Throwaway matmuls keep the PE busy from ~7us on: the HAM clock
        # gate needs ~3.4us of sustained PE activity to lift the 1.2GHz
        # cold throttle, so the conv stream starts at the full 2.4GHz
        # instead of paying a cold-ramp.  First batch before the delta MMs
        # (no DMA dependency at all), second batch after, filling the gap
        # until the conv weights are ready.
        warmP = wpsum.tile([128, 480], f32)
        for t in range(5):
            nc.tensor.matmul(
                warmP[:], warm0[:, 0:C], warm0[:], start=True, stop=True
            )

        # deltaT_scaled[i, t, o] = 0.25 * sum_r down[r,i,t] * up[o,r];
        # wsum = Wb^T + deltaT (unmodulated)
        deltaP = wpsum.tile([128, C9], f32)
        for t in range(9):
            nc.tensor.matmul(
                deltaP[:, t * C : (t + 1) * C],
                LOR[:, t * C : (t + 1) * C],
                LOR[:, 9 * C : 10 * C],
                start=True,
                stop=True,
            )
        for t in range(8):
            nc.tensor.matmul(
                warmP[:], warm0[:, 0:C], warm0[:], start=True, stop=True
            )
        nc.vector.tensor_add(wsum[:], W9[:, 0:C9], deltaP[:])
        nc.vector.tensor_scalar_mul(wm3[:], wsum[:], dmf[:, 0:1])

        # Combined-tap construction, ordered so phase-0's weights (wm3
        # views, cmb[0]A/B, R01 views, cmb[1]A/B) complete first and the
        # conv stream starts while the rest finish.  (8 of the 16 combined
        # taps are direct views into wm3/R01/R10; only column-sums are
        # materialized.)
        rcs = {
            (0, 0): (wm3, 0),
            (0, 1): (R01, 0),
            (1, 0): (R10, 0),
            (1, 1): (wm3, 6 * C),
        }
        nc.vector.tensor_add(R01[:], wm3[:, 3 * C : 6 * C], wm3[:, 6 * C : C9])

        def cmb_build(i, di, a):
            tl, base = rcs[(di, a)]
            nc.vector.tensor_add(
                cmb[:, i, 0, :],
                tl[:, base + C : base + 2 * C],
                tl[:, base + 2 * C : base + 3 * C],
            )
            nc.vector.tensor_add(
                cmb[:, i, 1, :],
                tl[:, base : base + C],
                tl[:, base + C : base + 2 * C],
            )

        cmb_build(0, 0, 0)
        cmb_build(1, 0, 1)
        nc.vector.tensor_add(R10[:], wm3[:, 0 : 3 * C], wm3[:, 3 * C : 6 * C])
        cmb_build(2, 1, 0)
        cmb_build(3, 1, 1)

        # demod: S = sum_t wsum^2 (ACT square + DVE add tree) then the
        # tiny N=1 matmul against dm^2.  This all runs INSIDE the weight
        # stage: the first eviction -- which opens the HBM output stream,
        # the closing critical path -- needs demP.
        sq3 = wtmp.tile([128, C9], bf16)
        nc.scalar.square(sq3[:], wsum[:])
        a4 = wtmp.tile([128, 4 * C], bf16)
        nc.vector.tensor_add(a4[:], sq3[:, 0 : 4 * C], sq3[:, 4 * C : 8 * C])
        a2 = wtmp.tile([128, 2 * C], bf16)
        nc.vector.tensor_add(a2[:], a4[:, 0 : 2 * C], a4[:, 2 * C : 4 * C])
        s2t = wtmp.tile([128, C], bf16)
        nc.vector.tensor_add(s2t[:], a2[:, 0:C], a2[:, C : 2 * C])
        s2 = wtmp.tile([128, C], bf16)
        nc.vector.tensor_add(s2[:], s2t[:], sq3[:, 8 * C : C9])

        sP = spsum.tile([128, 1], f32)
        nc.tensor.matmul(sP[:], s2[:], dm2[:], start=True, stop=True)
        t2 = wtmp.tile([128, 1], f32)
        nc.scalar.activation(t2[:], sP[:], AF.Sqrt, bias=epsA[:, 0:1])
        nc.vector.reciprocal(demP[:], t2[:])

    # wpsum (deltaP, warmP, sP) is closed here so the conv PSUM pool can
    # reuse its banks.

    def lhsT_ap(di, dj, a, b):
        tl, base = rcs[(di, a)]
        if dj == 0 and b == 0:
            return tl[:, base : base + C]
        if dj == 1 and b == 1:
            return tl[:, base + 2 * C : base + 3 * C]
        return cmb[:, di * 2 + a, 0 if dj == 0 else 1, :]

    # ---- main conv loop ----
    mpsum = ctx.enter_context(tc.tile_pool(name="mpsum", bufs=7, space="PSUM"))
    opool = ctx.enter_context(tc.tile_pool(name="obuf", bufs=8))

    def emit_mms(i0, R):
        bt, s = band_tiles[_band_of(i0)]
        ph = []
        for p in range(4):
            di, dj = p >> 1, p & 1
            pt = mpsum.tile([128, R * W], f32, tag="ph", name=f"ph{p}_{i0}")
            for q in range(4):
                a, b = q >> 1, q & 1
                r0 = i0 + a + di - s         # padded row within band tile
                rhs = bt[:, r0 : r0 + R, b + dj : b + dj + W]
                nc.tensor.matmul(
                    pt[:], lhsT_ap(di, dj, a, b), rhs,
                    start=(q == 0), stop=(q == 3),
                )
            ph.append(pt)
        return ph

    def emit_evict(i0, R, ph):
        # interleave phases into full output rows; scale by demod, add bias
        ob = opool.tile([128, R, 2, 2 * W], f32, tag="ob", name=f"ob_{i0}")
        obv = ob.rearrange("p r d (j two) -> p r d two j", two=2)
        for p in range(4):
            di, dj = p >> 1, p & 1
            dst = obv[:, :, di, dj, :]
            srcv = ph[p].rearrange("p (r j) -> p r j", r=R)
            if dj == 0:
                nc.vector.tensor_scalar(
                    dst, srcv, demP[:, 0:1], evb[:, 0:1],
                    op0=ALU.mult, op1=ALU.add,
                )
            else:
                nc.scalar.activation(
                    dst, srcv, AF.Identity, bias=evb[:, 0:1], scale=demP[:, 0:1]
                )
        # rotate the DMA queues so one queue's end-of-DMA completion
        # receipt (~1-2us) overlaps the others' data movement.  GpSimd
        # (SWDGE) joins the rotation once its band descgen chain is done.
        k = i0 // R_BLK
        if k < 10:
            eng = nc.sync if k % 2 == 0 else nc.scalar
        else:
            eng = (nc.sync, nc.scalar, nc.gpsimd)[k % 3]
        eng.dma_start(y[:, 2 * i0 : 2 * i0 + 2 * R, :], ob[:])

    # R=3 blocks for the body, four R=1 blocks last: the small final
    # output DMAs drain inline with the last matmuls instead of leaving a
    # ~3us backlog after the PE goes idle.  (No small blocks at the head:
    # early evictions are demP-gated anyway, and burning PSUM-pool slots
    # on quick small blocks just stalls the conv stream.)
    blocks = [(i0, R_BLK) for i0 in range(0, 156, R_BLK)]
    blocks += [(i0, 1) for i0 in range(156, H)]

    for i0, R in blocks:
        emit_evict(i0, R, emit_mms(i0, R))


def _build():
    nc = bacc.Bacc(
        "TRN2",
        target_bir_lowering=False,
        debug=False,
        enable_asserts=False,
        num_devices=NCORES,
    )
    x = nc.dram_tensor("x", [C, HP, WP], bf16, kind="ExternalInput").ap()
    wpk = nc.dram_tensor("wpk", [C, C9 + 2 + 10 * C], bf16, kind="ExternalInput").ap()
    y = nc.dram_tensor("y", [C, 2 * H, 2 * W], f32, kind="ExternalOutput").ap()

    with tile.TileContext(nc) as tc:
        with ExitStack() as ctx:
            _conv_kernel(ctx, tc, y, x, wpk)
    nc.compile()
    return nc


_CACHE = {}


def _get_nc():
    if "nc" not in _CACHE:
        _CACHE["nc"] = _build()
    return _CACHE["nc"]


def _make_in_maps(x, de_mod, Wb, lora_up, lora_down, bias):
    BF = mybir.dt.np(bf16)
    x = np.asarray(x, dtype=np.float32)
    de_mod = np.asarray(de_mod, dtype=np.float32)
    Wb = np.asarray(Wb, dtype=np.float32)
    lora_up = np.asarray(lora_up, dtype=np.float32)
    lora_down = np.asarray(lora_down, dtype=np.float32)
    bias = np.asarray(bias, dtype=np.float32).reshape(C)

    # zero-pad x with a 1-px border; bf16
    xp = np.zeros((B, C, HP, WP), dtype=BF)
    xp[:, :, 1 : 1 + H, 1 : 1 + W] = x.astype(BF)

    # [O,I,3,3] -> [i, (t o)];  [R,C,3,3] -> [r, (t i)]
    wbT = np.ascontiguousarray(Wb.transpose(1, 2, 3, 0).reshape(C, C9))
    ld = lora_down.transpose(0, 2, 3, 1).reshape(RANK, C9)
    lu = SCALING * lora_up.T                    # [r, o], lora scale folded in
    lor = np.concatenate([ld, lu], axis=1)

    in_maps = []
    for b in range(NCORES):
        wpk = np.zeros((C, C9 + 2 + 10 * C), dtype=np.float32)
        wpk[:, 0:C9] = wbT
        wpk[:, C9] = de_mod[b]
        wpk[:, C9 + 1] = bias
        wpk[0:RANK, C9 + 2 :] = lor
        in_maps.append(
            {
                "x": np.ascontiguousarray(xp[b]),
                "wpk": wpk.astype(BF),
            }
        )
    return in_maps


def run(inputs, trace=False, trace_kwargs=None):
    nc = _get_nc()
    in_maps = _make_in_maps(**inputs)
    res = run_bass_kernel_spmd(
        nc,
        in_maps,
        core_ids=list(range(NCORES)),
        trace=trace,
        **(trace_kwargs or {}),
    )
    y = np.stack([res.results[b]["y"] for b in range(NCORES)], axis=0)
    return y, res


def kernel(**inputs):
    y, _ = run(inputs)
    return y


# revision 30
# speedup vs baseline: 1.0083x; 1.0083x over previous
"""Trainium2 Bass kernel for nn_NeuronS3DiffUpsample2D.

Reference computation (per sample b):
    up   = nearest-2x-upsample(x[b])                       # [C, 320, 320]
    w    = Wb + 0.25 * einsum('or,rikl->oikl', lora_up, lora_down)
    w_b  = w * de_mod[b, None, :, None, None]              # modulate input chans
    dem  = rsqrt(sum_{i,k,l} w_b^2 + eps)                  # per output chan
    y[b] = conv2d(up, w_b * dem, SAME) + bias

Key algebraic transform: a 3x3 SAME conv on a 2x nearest-upsampled image
decomposes into 4 output phases (di, dj in {0,1}), each a 2x2 conv on the
ORIGINAL 160x160 input:
    y[2i+di, 2j+dj] = sum_{a,b in {0,1}} K[di,dj,a,b] @ x[i+a+di-1, j+b+dj-1]
where each K[di,dj,a,b] is a row-combo x col-combo sum of the 9 taps of w:
  row-combos (di,a): {w0, w1+w2, w0+w1, w2} over ki; same pattern over kj.
This is 4/9 of the naive FLOPs and never materializes the upsampled image.

Since the demod scale is per output channel and conv is linear in w, the conv
OUTPUT is scaled by dem[o] (per-partition scalar) at PSUM eviction, fused with
the bias add; weights are only modulated by de_mod on the input-channel axis.

Sharding: data-parallel over batch B=8 across 8 NeuronCores; each core builds
its own per-sample weights locally (replicated W/lora are tiny).

Performance notes (from perfetto traces of earlier revisions):
  * The conv loop is a zero-gap matmul stream; its cadence was set by f32r
    LDWEIGHTS (224 ns > the 200 ns N=480 matmul).  All matmul operands are
    bf16 now: LDWEIGHTS takes ~107 ns (with FWL) and hides fully, and the
    input DMA bytes halve.  Accumulation stays fp32 in PSUM; rel err ~2e-3
    against the fp32 reference.
  * x is padded to [C,162,162] with a zero border ON HOST so every band DMA
    is a single contiguous descriptor per partition (no SWDGE descriptor
    storms, no DVE border memsets) and arrives fast.
  * Of the 16 combined-tap matrices, 8 are direct views into the row-combo
    tiles (no copies); only the 8 column-sums are materialized by DVE.
  * The demod reduction uses 4 contiguous DVE adds instead of one strided
    tensor_reduce; its tiny PE matmul is scheduled before the conv stream so
    the PSUM pool for the conv loop can own all 8 banks.
"""

import sys
import numpy as np
from contextlib import ExitStack

try:
    import concourse.bass as bass
except ImportError:  # grading env without the axon PYTHONPATH
    sys.path.insert(0, "/opt/trn_rl_repo")
    import concourse.bass as bass
import concourse.tile as tile
from concourse import bacc, mybir
from concourse.bass_utils import run_bass_kernel_spmd

B, C, H, W = 8, 128, 160, 160
RANK = 32
SCALING = 0.25
EPS = 1e-8
HP, WP = H + 2, W + 2   # zero-padded image (1-px border baked in on host)
R_BLK = 3               # x-rows per matmul block -> N = 3*160 = 480 <= 512
C9 = 9 * C
NCORES = 8

# Input bands (padded-row ranges).  Block i0 needs padded rows [i0, i0+4];
# bands overlap by 4 rows so any block reads from a single tile.  The first
# band is small so the conv stream can start as soon as the weight stage is
# done; later bands are large to amortize DMA setup.
BANDS = [(0, 14), (12, 26), (24, 50), (48, 86), (84, 124), (120, 162)]

f32 = mybir.dt.float32
bf16 = mybir.dt.bfloat16


def _band_of(i0):
    if i0 <= 9:
        return 0
    if i0 <= 21:
        return 1
    if i0 <= 45:
        return 2
    if i0 <= 81:
        return 3
    if i0 <= 117:
        return 4
    return 5


def _conv_kernel(ctx, tc, y, x, wpk):
    nc = tc.nc
    AF = mybir.ActivationFunctionType
    ALU = mybir.AluOpType

    const = ctx.enter_context(tc.tile_pool(name="const", bufs=1))

    demP = const.tile([128, 1], f32)         # rsqrt demod, per output chan
    evb = const.tile([128, 1], f32)          # bias[o], f32 for evictions
    dmf = const.tile([128, 1], f32)          # de_mod[i], f32 scalar operand
    wm3 = const.tile([128, C9], bf16)        # modulated 9-tap weights [i,(t o)]
    R01 = const.tile([128, 3 * C], bf16)     # row-combo ki1+ki2
    R10 = const.tile([128, 3 * C], bf16)     # row-combo ki0+ki1
    cmb = const.tile([128, 4, 2, C], bf16)   # col-sums per (di,a): [A=kj1+kj2, B=kj0+kj1]
    # Wb^T [i,(t o)] + de_mod col + bias col + (rows 0-31) lora pack.  One
    # tensor -> ONE weight DMA: a second DMA on the same HWDGE ring pays a
    # ~4us completion penalty that would gate the whole weight stage.
    W9 = const.tile([128, C9 + 2 + 10 * C], bf16)

    # x bands: contiguous 1-descriptor-per-partition DMAs on the otherwise
    # idle GpSimd queue (separate from the weight DMAs on sync and the
    # output DMAs on sync).  band0 is issued immediately; bands 1-4 are
    # held behind a probe op that depends on the W9 weight DMA so their
    # bulk transfers don't steal SDMA engines from the weight stage.
    band_tiles = []
    for bi, (s, e) in enumerate(BANDS):
        bt = const.tile([128, e - s, WP], bf16, name=f"band{bi}")
        band_tiles.append((bt, s))

    dmv = W9[:, C9 : C9 + 1]                 # de_mod[i] per partition
    biasv = W9[:, C9 + 1 : C9 + 2]

    wtmp = ctx.enter_context(tc.tile_pool(name="wtmp", bufs=1))
    spsum = ctx.enter_context(tc.tile_pool(name="spsum", bufs=1, space="PSUM"))
    with tc.tile_pool(name="wpsum", bufs=1, space="PSUM") as wpsum:
        nc.sync.dma_start(W9[:], wpk[:])
        LOR = W9[0:RANK, C9 + 2 : C9 + 2 + 10 * C]   # [lora_down^T | 0.25*lora_up^T]

        # Bands are chained: band k+1's descriptor generation waits (via a
        # 1-element probe) for band k's completion.  This pins the transfer
        # order band0 < band1 < ... (the scheduler otherwise reorders the
        # descgens and a late band stalls the conv stream) and keeps early
        # band traffic from flooding the SDMA engines all at once.
        for bi in range(len(BANDS)):
            bt, s = band_tiles[bi]
            nc.gpsimd.dma_start(bt[:], x[:, s : BANDS[bi][1], :])
            if bi + 1 < len(BANDS):
                bprobe = wtmp.tile([1, 1], bf16, name=f"bprobe{bi}")
                nc.gpsimd.tensor_copy(bprobe[:], bt[0:1, 0, 0:1])
        wsum = wtmp.tile([128, C9], bf16)

        # DMA-independent PE warm-up fuel: the first warm matmuls must not
        # wait for any DMA, so the HAM activity window starts filling at
        # ~7us and the clock gate is open before the conv stream begins.
        warm0 = wtmp.tile([128, 480], bf16)
        nc.vector.memset(warm0[:], 0.25)
        epsA = wtmp.tile([128, 1], f32)
        nc.vector.memset(epsA[:], EPS)
        nc.vector.tensor_copy(evb[:], biasv)
        nc.vector.tensor_copy(dmf[:], dmv)
        dm2 = wtmp.tile([128, 1], bf16)          # de_mod[i]^2, demod matmul rhs
        nc.scalar.square(dm2[:], dmf[:])

        # Throwaway matmuls keep the PE busy from ~7us on: the HAM clock
        # gate needs ~3.4us of sustained PE activity to lift the 1.2GHz
        # cold throttle, so the conv stream starts at the full 2.4GHz
        # instead of paying a cold-ramp.  First batch before the delta MMs
        # (no DMA dependency at all), second batch after, filling the gap
        # until the conv weights are ready.
        warmP = wpsum.tile([128, 480], f32)
        for t in range(5):
            nc.tensor.matmul(
                warmP[:], warm0[:, 0:C], warm0[:], start=True, stop=True
            )

        # deltaT_scaled[i, t, o] = 0.25 * sum_r down[r,i,t] * up[o,r];
        # wsum = Wb^T + deltaT (unmodulated)
        deltaP = wpsum.tile([128, C9], f32)
        for t in range(9):
            nc.tensor.matmul(
                deltaP[:, t * C : (t + 1) * C],
                LOR[:, t * C : (t + 1) * C],
                LOR[:, 9 * C : 10 * C],
                start=True,
                stop=True,
            )
        for t in range(8):
            nc.tensor.matmul(
                warmP[:], warm0[:, 0:C], warm0[:], start=True, stop=True
            )
        nc.vector.tensor_add(wsum[:], W9[:, 0:C9], deltaP[:])
        nc.vector.tensor_scalar_mul(wm3[:], wsum[:], dmf[:, 0:1])

        # Combined-tap construction, ordered so phase-0's weights (wm3
        # views, cmb[0]A/B, R01 views, cmb[1]A/B) complete first and the
        # conv stream starts while the rest finish.  (8 of the 16 combined
        # taps are direct views into wm3/R01/R10; only column-sums are
        # materialized.)
        rcs = {
            (0, 0): (wm3, 0),
            (0, 1): (R01, 0),
            (1, 0): (R10, 0),
            (1, 1): (wm3, 6 * C),
        }
        nc.vector.tensor_add(R01[:], wm3[:, 3 * C : 6 * C], wm3[:, 6 * C : C9])

        def cmb_build(i, di, a):
            tl, base = rcs[(di, a)]
            nc.vector.tensor_add(
                cmb[:, i, 0, :],
                tl[:, base + C : base + 2 * C],
                tl[:, base + 2 * C : base + 3 * C],
            )
            nc.vector.tensor_add(
                cmb[:, i, 1, :],
                tl[:, base : base + C],
                tl[:, base + C : base + 2 * C],
            )

        cmb_build(0, 0, 0)
        cmb_build(1, 0, 1)
        nc.vector.tensor_add(R10[:], wm3[:, 0 : 3 * C], wm3[:, 3 * C : 6 * C])
        cmb_build(2, 1, 0)
        cmb_build(3, 1, 1)

        # demod: S = sum_t wsum^2 (ACT square + DVE add tree) then the
        # tiny N=1 matmul against dm^2.  This all runs INSIDE the weight
        # stage: the first eviction -- which opens the HBM output stream,
        # the closing critical path -- needs demP.
        sq3 = wtmp.tile([128, C9], bf16)
        nc.scalar.square(sq3[:], wsum[:])
        a4 = wtmp.tile([128, 4 * C], bf16)
        nc.vector.tensor_add(a4[:], sq3[:, 0 : 4 * C], sq3[:, 4 * C : 8 * C])
        a2 = wtmp.tile([128, 2 * C], bf16)
        nc.vector.tensor_add(a2[:], a4[:, 0 : 2 * C], a4[:, 2 * C : 4 * C])
        s2t = wtmp.tile([128, C], bf16)
        nc.vector.tensor_add(s2t[:], a2[:, 0:C], a2[:, C : 2 * C])
        s2 = wtmp.tile([128, C], bf16)
        nc.vector.tensor_add(s2[:], s2t[:], sq3[:, 8 * C : C9])


    # wpsum (deltaP, warmP, sP) is closed here so the conv PSUM pool can
    # reuse its banks.

    def lhsT_ap(di, dj, a, b):
        tl, base = rcs[(di, a)]
        if dj == 0 and b == 0:
            return tl[:, base : base + C]
        if dj == 1 and b == 1:
            return tl[:, base + 2 * C : base + 3 * C]
        return cmb[:, di * 2 + a, 0 if dj == 0 else 1, :]

    # ---- main conv loop ----
    mpsum = ctx.enter_context(tc.tile_pool(name="mpsum", bufs=7, space="PSUM"))
    opool = ctx.enter_context(tc.tile_pool(name="obuf", bufs=8))

    def emit_mms(i0, R):
        bt, s = band_tiles[_band_of(i0)]
        ph = []
        for p in range(4):
            di, dj = p >> 1, p & 1
            pt = mpsum.tile([128, R * W], f32, tag="ph", name=f"ph{p}_{i0}")
            for q in range(4):
                a, b = q >> 1, q & 1
                r0 = i0 + a + di - s         # padded row within band tile
                rhs = bt[:, r0 : r0 + R, b + dj : b + dj + W]
                nc.tensor.matmul(
                    pt[:], lhsT_ap(di, dj, a, b), rhs,
                    start=(q == 0), stop=(q == 3),
                )
            ph.append(pt)
        return ph

    def emit_evict(i0, R, ph):
        # interleave phases into full output rows; scale by demod, add bias
        ob = opool.tile([128, R, 2, 2 * W], f32, tag="ob", name=f"ob_{i0}")
        obv = ob.rearrange("p r d (j two) -> p r d two j", two=2)
        for p in range(4):
            di, dj = p >> 1, p & 1
            dst = obv[:, :, di, dj, :]
            srcv = ph[p].rearrange("p (r j) -> p r j", r=R)
            if dj == 0:
                nc.vector.tensor_scalar(
                    dst, srcv, demP[:, 0:1], evb[:, 0:1],
                    op0=ALU.mult, op1=ALU.add,
                )
            else:
                nc.scalar.activation(
                    dst, srcv, AF.Identity, bias=evb[:, 0:1], scale=demP[:, 0:1]
                )
        # rotate the DMA queues so one queue's end-of-DMA completion
        # receipt (~1-2us) overlaps the others' data movement.  GpSimd
        # (SWDGE) joins the rotation once its band descgen chain is done.
        k = i0 // R_BLK
        if k < 10:
            eng = nc.sync if k % 2 == 0 else nc.scalar
        else:
            eng = (nc.sync, nc.scalar, nc.gpsimd)[k % 3]
        eng.dma_start(y[:, 2 * i0 : 2 * i0 + 2 * R, :], ob[:])

    # R=3 blocks for the body, four R=1 blocks last: the small final
    # output DMAs drain inline with the last matmuls instead of leaving a
    # ~3us backlog after the PE goes idle.  (No small blocks at the head:
    # early evictions are demP-gated anyway, and burning PSUM-pool slots
    # on quick small blocks just stalls the conv stream.)
    blocks = [(i0, R_BLK) for i0 in range(0, 156, R_BLK)]
    blocks += [(i0, 1) for i0 in range(156, H)]

    # demod matmul sits AFTER block0's matmuls in the PE queue: the conv
    # stream starts as soon as the combined-tap weights land, without
    # waiting for the demod reduce chain (block0's eviction still waits
    # for demP, but the PSUM pool gives ~2 blocks of slack).
    ph0 = emit_mms(*blocks[0])
    sP = spsum.tile([128, 1], f32)
    nc.tensor.matmul(sP[:], s2[:], dm2[:], start=True, stop=True)
    t2 = wtmp.tile([128, 1], f32)
    nc.scalar.activation(t2[:], sP[:], AF.Sqrt, bias=epsA[:, 0:1])
    nc.vector.reciprocal(demP[:], t2[:])
    emit_evict(*blocks[0], ph0)
    for i0, R in blocks[1:]:
        emit_evict(i0, R, emit_mms(i0, R))


def _build():
    nc = bacc.Bacc(
        "TRN2",
        target_bir_lowering=False,
        debug=False,
        enable_asserts=False,
        num_devices=NCORES,
    )
    x = nc.dram_tensor("x", [C, HP, WP], bf16, kind="ExternalInput").ap()
    wpk = nc.dram_tensor("wpk", [C, C9 + 2 + 10 * C], bf16, kind="ExternalInput").ap()
    y = nc.dram_tensor("y", [C, 2 * H, 2 * W], f32, kind="ExternalOutput").ap()

    with tile.TileContext(nc) as tc:
        with ExitStack() as ctx:
            _conv_kernel(ctx, tc, y, x, wpk)
    nc.compile()
    return nc


_CACHE = {}


def _get_nc():
    if "nc" not in _CACHE:
        _CACHE["nc"] = _build()
    return _CACHE["nc"]


def _make_in_maps(x, de_mod, Wb, lora_up, lora_down, bias):
    BF = mybir.dt.np(bf16)
    x = np.asarray(x, dtype=np.float32)
    de_mod = np.asarray(de_mod, dtype=np.float32)
    Wb = np.asarray(Wb, dtype=np.float32)
    lora_up = np.asarray(lora_up, dtype=np.float32)
    lora_down = np.asarray(lora_down, dtype=np.float32)
    bias = np.asarray(bias, dtype=np.float32).reshape(C)

    # zero-pad x with a 1-px border; bf16
    xp = np.zeros((B, C, HP, WP), dtype=BF)
    xp[:, :, 1 : 1 + H, 1 : 1 + W] = x.astype(BF)

    # [O,I,3,3] -> [i, (t o)];  [R,C,3,3] -> [r, (t i)]
    wbT = np.ascontiguousarray(Wb.transpose(1, 2, 3, 0).reshape(C, C9))
    ld = lora_down.transpose(0, 2, 3, 1).reshape(RANK, C9)
    lu = SCALING * lora_up.T                    # [r, o], lora scale folded in
    lor = np.concatenate([ld, lu], axis=1)

    in_maps = []
    for b in range(NCORES):
        wpk = np.zeros((C, C9 + 2 + 10 * C), dtype=np.float32)
        wpk[:, 0:C9] = wbT
        wpk[:, C9] = de_mod[b]
        wpk[:, C9 + 1] = bias
        wpk[0:RANK, C9 + 2 :] = lor
        in_maps.append(
            {
                "x": np.ascontiguousarray(xp[b]),
                "wpk": wpk.astype(BF),
            }
        )
    return in_maps


def run(inputs, trace=False, trace_kwargs=None):
    nc = _get_nc()
    in_maps = _make_in_maps(**inputs)
    res = run_bass_kernel_spmd(
        nc,
        in_maps,
        core_ids=list(range(NCORES)),
        trace=trace,
        **(trace_kwargs or {}),
    )
    y = np.stack([res.results[b]["y"] for b in range(NCORES)], axis=0)
    return y, res


def kernel(**inputs):
    y, _ = run(inputs)
    return y


# revision 31
# speedup vs baseline: 1.0141x; 1.0057x over previous
"""Trainium2 Bass kernel for nn_NeuronS3DiffUpsample2D.

Reference computation (per sample b):
    up   = nearest-2x-upsample(x[b])                       # [C, 320, 320]
    w    = Wb + 0.25 * einsum('or,rikl->oikl', lora_up, lora_down)
    w_b  = w * de_mod[b, None, :, None, None]              # modulate input chans
    dem  = rsqrt(sum_{i,k,l} w_b^2 + eps)                  # per output chan
    y[b] = conv2d(up, w_b * dem, SAME) + bias

Key algebraic transform: a 3x3 SAME conv on a 2x nearest-upsampled image
decomposes into 4 output phases (di, dj in {0,1}), each a 2x2 conv on the
ORIGINAL 160x160 input:
    y[2i+di, 2j+dj] = sum_{a,b in {0,1}} K[di,dj,a,b] @ x[i+a+di-1, j+b+dj-1]
where each K[di,dj,a,b] is a row-combo x col-combo sum of the 9 taps of w:
  row-combos (di,a): {w0, w1+w2, w0+w1, w2} over ki; same pattern over kj.
This is 4/9 of the naive FLOPs and never materializes the upsampled image.

Since the demod scale is per output channel and conv is linear in w, the conv
OUTPUT is scaled by dem[o] (per-partition scalar) at PSUM eviction, fused with
the bias add; weights are only modulated by de_mod on the input-channel axis.

Sharding: data-parallel over batch B=8 across 8 NeuronCores; each core builds
its own per-sample weights locally (replicated W/lora are tiny).

Performance notes (from perfetto traces of earlier revisions; exec time on
8 cores went 225us -> ~199us over these steps):
  * The conv loop is a zero-gap matmul stream; with f32r operands its
    cadence was set by LDWEIGHTS (224 ns > the 200 ns N=480 matmul).  All
    matmul operands are bf16: LDWEIGHTS (~97 ns, FWL) hides fully and the
    input DMA bytes halve.  Accumulation stays fp32 in PSUM; rel err ~4e-3
    against the fp32 reference (gate is 2e-2).
  * x is padded to [C,162,162] with a zero border ON HOST so every band DMA
    is a single contiguous descriptor per partition (no SWDGE descriptor
    storms, no DVE border memsets).  Bands are chained via 1-element probe
    ops so their transfer order is pinned and they never starve the stream.
  * ALL weights ride ONE sync-ring DMA (wpk packs Wb^T, de_mod, bias, and
    the lora operands): a second DMA on the same HWDGE ring was measured to
    complete its semaphore ~4us later than the first.
  * Of the 16 combined-tap matrices, 8 are direct views into the row-combo
    tiles (no copies); only the 8 column-sums are materialized by DVE, in
    phase-0-first order so the conv stream starts as early as possible.
  * The demod reduction is an ACT square + 4 contiguous DVE adds (a strided
    tensor_reduce is ~4x slower); the i-reduction is an N=1 matmul with
    dm^2 as the moving operand, scheduled AFTER block0's matmuls so conv
    start is not gated on it; sqrt(x+eps) is a single ACT op (bias=eps AP).
  * Throwaway matmuls on a memset tile keep the PE busy from ~7us so the
    HAM clock gate (1.2 -> 2.4 GHz after ~3.4us of activity) is open when
    the conv stream starts.
  * The closing critical path is the HBM output stream (52 MB of y).  Output
    DMAs rotate across the sync/scalar/gpsimd queues so one queue's
    completion receipt overlaps the others' data; obuf bufs=8 rides the
    transients; the last four row-blocks are R=1 so the final DMAs are
    small and drain inline with the last matmuls.
"""

import sys
import numpy as np
from contextlib import ExitStack

try:
    import concourse.bass as bass
except ImportError:  # grading env without the axon PYTHONPATH
    sys.path.insert(0, "/opt/trn_rl_repo")
    import concourse.bass as bass
import concourse.tile as tile
from concourse import bacc, mybir
from concourse.bass_utils import run_bass_kernel_spmd

B, C, H, W = 8, 128, 160, 160
RANK = 32
SCALING = 0.25
EPS = 1e-8
HP, WP = H + 2, W + 2   # zero-padded image (1-px border baked in on host)
R_BLK = 3               # x-rows per matmul block -> N = 3*160 = 480 <= 512
C9 = 9 * C
NCORES = 8

# Input bands (padded-row ranges).  Block i0 needs padded rows [i0, i0+4];
# bands overlap by 4 rows so any block reads from a single tile.  The first
# band is small so the conv stream can start as soon as the weight stage is
# done; later bands are large to amortize DMA setup.
BANDS = [(0, 14), (12, 26), (24, 50), (48, 86), (84, 124), (120, 162)]

f32 = mybir.dt.float32
bf16 = mybir.dt.bfloat16


def _band_of(i0):
    if i0 <= 9:
        return 0
    if i0 <= 21:
        return 1
    if i0 <= 45:
        return 2
    if i0 <= 81:
        return 3
    if i0 <= 117:
        return 4
    return 5


def _conv_kernel(ctx, tc, y, x, wpk):
    nc = tc.nc
    AF = mybir.ActivationFunctionType
    ALU = mybir.AluOpType

    const = ctx.enter_context(tc.tile_pool(name="const", bufs=1))

    demP = const.tile([128, 1], f32)         # rsqrt demod, per output chan
    evb = const.tile([128, 1], f32)          # bias[o], f32 for evictions
    dmf = const.tile([128, 1], f32)          # de_mod[i], f32 scalar operand
    wm3 = const.tile([128, C9], bf16)        # modulated 9-tap weights [i,(t o)]
    R01 = const.tile([128, 3 * C], bf16)     # row-combo ki1+ki2
    R10 = const.tile([128, 3 * C], bf16)     # row-combo ki0+ki1
    cmb = const.tile([128, 4, 2, C], bf16)   # col-sums per (di,a): [A=kj1+kj2, B=kj0+kj1]
    # Wb^T [i,(t o)] + de_mod col + bias col + (rows 0-31) lora pack.  One
    # tensor -> ONE weight DMA: a second DMA on the same HWDGE ring pays a
    # ~4us completion penalty that would gate the whole weight stage.
    W9 = const.tile([128, C9 + 2 + 10 * C], bf16)

    # x bands: contiguous 1-descriptor-per-partition DMAs on the otherwise
    # idle GpSimd queue (separate from the weight DMAs on sync and the
    # output DMAs on sync).  band0 is issued immediately; bands 1-4 are
    # held behind a probe op that depends on the W9 weight DMA so their
    # bulk transfers don't steal SDMA engines from the weight stage.
    band_tiles = []
    for bi, (s, e) in enumerate(BANDS):
        bt = const.tile([128, e - s, WP], bf16, name=f"band{bi}")
        band_tiles.append((bt, s))

    dmv = W9[:, C9 : C9 + 1]                 # de_mod[i] per partition
    biasv = W9[:, C9 + 1 : C9 + 2]

    wtmp = ctx.enter_context(tc.tile_pool(name="wtmp", bufs=1))
    spsum = ctx.enter_context(tc.tile_pool(name="spsum", bufs=1, space="PSUM"))
    with tc.tile_pool(name="wpsum", bufs=1, space="PSUM") as wpsum:
        nc.sync.dma_start(W9[:], wpk[:])
        LOR = W9[0:RANK, C9 + 2 : C9 + 2 + 10 * C]   # [lora_down^T | 0.25*lora_up^T]

        # Bands are chained: band k+1's descriptor generation waits (via a
        # 1-element probe) for band k's completion.  This pins the transfer
        # order band0 < band1 < ... (the scheduler otherwise reorders the
        # descgens and a late band stalls the conv stream) and keeps early
        # band traffic from flooding the SDMA engines all at once.
        for bi in range(len(BANDS)):
            bt, s = band_tiles[bi]
            nc.gpsimd.dma_start(bt[:], x[:, s : BANDS[bi][1], :])
            if bi + 1 < len(BANDS):
                bprobe = wtmp.tile([1, 1], bf16, name=f"bprobe{bi}")
                nc.gpsimd.tensor_copy(bprobe[:], bt[0:1, 0, 0:1])
        wsum = wtmp.tile([128, C9], bf16)

        # DMA-independent PE warm-up fuel: the first warm matmuls must not
        # wait for any DMA, so the HAM activity window starts filling at
        # ~7us and the clock gate is open before the conv stream begins.
        warm0 = wtmp.tile([128, 480], bf16)
        nc.vector.memset(warm0[:], 0.25)
        epsA = wtmp.tile([128, 1], f32)
        nc.vector.memset(epsA[:], EPS)
        nc.vector.tensor_copy(evb[:], biasv)
        nc.vector.tensor_copy(dmf[:], dmv)
        dm2 = wtmp.tile([128, 1], bf16)          # de_mod[i]^2, demod matmul rhs
        nc.scalar.square(dm2[:], dmf[:])

        # Throwaway matmuls keep the PE busy from ~7us on: the HAM clock
        # gate needs ~3.4us of sustained PE activity to lift the 1.2GHz
        # cold throttle, so the conv stream starts at the full 2.4GHz
        # instead of paying a cold-ramp.  First batch before the delta MMs
        # (no DMA dependency at all), second batch after, filling the gap
        # until the conv weights are ready.
        warmP = wpsum.tile([128, 480], f32)
        for t in range(5):
            nc.tensor.matmul(
                warmP[:], warm0[:, 0:C], warm0[:], start=True, stop=True
            )

        # deltaT_scaled[i, t, o] = 0.25 * sum_r down[r,i,t] * up[o,r];
        # wsum = Wb^T + deltaT (unmodulated)
        deltaP = wpsum.tile([128, C9], f32)
        for t in range(9):
            nc.tensor.matmul(
                deltaP[:, t * C : (t + 1) * C],
                LOR[:, t * C : (t + 1) * C],
                LOR[:, 9 * C : 10 * C],
                start=True,
                stop=True,
            )
        for t in range(8):
            nc.tensor.matmul(
                warmP[:], warm0[:, 0:C], warm0[:], start=True, stop=True
            )
        nc.vector.tensor_add(wsum[:], W9[:, 0:C9], deltaP[:])
        nc.vector.tensor_scalar_mul(wm3[:], wsum[:], dmf[:, 0:1])

        # Combined-tap construction, ordered so phase-0's weights (wm3
        # views, cmb[0]A/B, R01 views, cmb[1]A/B) complete first and the
        # conv stream starts while the rest finish.  (8 of the 16 combined
        # taps are direct views into wm3/R01/R10; only column-sums are
        # materialized.)
        rcs = {
            (0, 0): (wm3, 0),
            (0, 1): (R01, 0),
            (1, 0): (R10, 0),
            (1, 1): (wm3, 6 * C),
        }
        nc.vector.tensor_add(R01[:], wm3[:, 3 * C : 6 * C], wm3[:, 6 * C : C9])

        def cmb_build(i, di, a):
            tl, base = rcs[(di, a)]
            nc.vector.tensor_add(
                cmb[:, i, 0, :],
                tl[:, base + C : base + 2 * C],
                tl[:, base + 2 * C : base + 3 * C],
            )
            nc.vector.tensor_add(
                cmb[:, i, 1, :],
                tl[:, base : base + C],
                tl[:, base + C : base + 2 * C],
            )

        cmb_build(0, 0, 0)
        cmb_build(1, 0, 1)
        nc.vector.tensor_add(R10[:], wm3[:, 0 : 3 * C], wm3[:, 3 * C : 6 * C])
        cmb_build(2, 1, 0)
        cmb_build(3, 1, 1)

        # demod: S = sum_t wsum^2 (ACT square + DVE add tree) then the
        # tiny N=1 matmul against dm^2.  This all runs INSIDE the weight
        # stage: the first eviction -- which opens the HBM output stream,
        # the closing critical path -- needs demP.
        sq3 = wtmp.tile([128, C9], bf16)
        nc.scalar.square(sq3[:], wsum[:])
        a4 = wtmp.tile([128, 4 * C], bf16)
        nc.vector.tensor_add(a4[:], sq3[:, 0 : 4 * C], sq3[:, 4 * C : 8 * C])
        a2 = wtmp.tile([128, 2 * C], bf16)
        nc.vector.tensor_add(a2[:], a4[:, 0 : 2 * C], a4[:, 2 * C : 4 * C])
        s2t = wtmp.tile([128, C], bf16)
        nc.vector.tensor_add(s2t[:], a2[:, 0:C], a2[:, C : 2 * C])
        s2 = wtmp.tile([128, C], bf16)
        nc.vector.tensor_add(s2[:], s2t[:], sq3[:, 8 * C : C9])


    # wpsum (deltaP, warmP, sP) is closed here so the conv PSUM pool can
    # reuse its banks.

    def lhsT_ap(di, dj, a, b):
        tl, base = rcs[(di, a)]
        if dj == 0 and b == 0:
            return tl[:, base : base + C]
        if dj == 1 and b == 1:
            return tl[:, base + 2 * C : base + 3 * C]
        return cmb[:, di * 2 + a, 0 if dj == 0 else 1, :]

    # ---- main conv loop ----
    mpsum = ctx.enter_context(tc.tile_pool(name="mpsum", bufs=7, space="PSUM"))
    opool = ctx.enter_context(tc.tile_pool(name="obuf", bufs=8))

    def emit_mms(i0, R):
        bt, s = band_tiles[_band_of(i0)]
        ph = []
        for p in range(4):
            di, dj = p >> 1, p & 1
            pt = mpsum.tile([128, R * W], f32, tag="ph", name=f"ph{p}_{i0}")
            for q in range(4):
                a, b = q >> 1, q & 1
                r0 = i0 + a + di - s         # padded row within band tile
                rhs = bt[:, r0 : r0 + R, b + dj : b + dj + W]
                nc.tensor.matmul(
                    pt[:], lhsT_ap(di, dj, a, b), rhs,
                    start=(q == 0), stop=(q == 3),
                )
            ph.append(pt)
        return ph

    def emit_evict(i0, R, ph):
        # interleave phases into full output rows; scale by demod, add bias
        ob = opool.tile([128, R, 2, 2 * W], f32, tag="ob", name=f"ob_{i0}")
        obv = ob.rearrange("p r d (j two) -> p r d two j", two=2)
        for p in range(4):
            di, dj = p >> 1, p & 1
            dst = obv[:, :, di, dj, :]
            srcv = ph[p].rearrange("p (r j) -> p r j", r=R)
            if dj == 0:
                nc.vector.tensor_scalar(
                    dst, srcv, demP[:, 0:1], evb[:, 0:1],
                    op0=ALU.mult, op1=ALU.add,
                )
            else:
                nc.scalar.activation(
                    dst, srcv, AF.Identity, bias=evb[:, 0:1], scale=demP[:, 0:1]
                )
        # rotate the DMA queues so one queue's end-of-DMA completion
        # receipt (~1-2us) overlaps the others' data movement.  GpSimd
        # (SWDGE) joins the rotation once its band descgen chain is done.
        k = i0 // R_BLK
        if k < 10:
            eng = nc.sync if k % 2 == 0 else nc.scalar
        else:
            eng = (nc.sync, nc.scalar, nc.gpsimd)[k % 3]
        eng.dma_start(y[:, 2 * i0 : 2 * i0 + 2 * R, :], ob[:])

    # R=3 blocks for the body, four R=1 blocks last: the small final
    # output DMAs drain inline with the last matmuls instead of leaving a
    # ~3us backlog after the PE goes idle.  (No small blocks at the head:
    # early evictions are demP-gated anyway, and burning PSUM-pool slots
    # on quick small blocks just stalls the conv stream.)
    blocks = [(i0, R_BLK) for i0 in range(0, 156, R_BLK)]
    blocks += [(i0, 1) for i0 in range(156, H)]

    # demod matmul sits AFTER block0's matmuls in the PE queue: the conv
    # stream starts as soon as the combined-tap weights land, without
    # waiting for the demod reduce chain (block0's eviction still waits
    # for demP, but the PSUM pool gives ~2 blocks of slack).
    ph0 = emit_mms(*blocks[0])
    sP = spsum.tile([128, 1], f32)
    nc.tensor.matmul(sP[:], s2[:], dm2[:], start=True, stop=True)
    t2 = wtmp.tile([128, 1], f32)
    nc.scalar.activation(t2[:], sP[:], AF.Sqrt, bias=epsA[:, 0:1])
    nc.vector.reciprocal(demP[:], t2[:])
    emit_evict(*blocks[0], ph0)
    for i0, R in blocks[1:]:
        emit_evict(i0, R, emit_mms(i0, R))


def _build():
    nc = bacc.Bacc(
        "TRN2",
        target_bir_lowering=False,
        debug=False,
        enable_asserts=False,
        num_devices=NCORES,
    )
    x = nc.dram_tensor("x", [C, HP, WP], bf16, kind="ExternalInput").ap()
    wpk = nc.dram_tensor("wpk", [C, C9 + 2 + 10 * C], bf16, kind="ExternalInput").ap()
    y = nc.dram_tensor("y", [C, 2 * H, 2 * W], f32, kind="ExternalOutput").ap()

    with tile.TileContext(nc) as tc:
        with ExitStack() as ctx:
            _conv_kernel(ctx, tc, y, x, wpk)
    nc.compile()
    return nc


_CACHE = {}


def _get_nc():
    if "nc" not in _CACHE:
        _CACHE["nc"] = _build()
    return _CACHE["nc"]


def _make_in_maps(x, de_mod, Wb, lora_up, lora_down, bias):
    BF = mybir.dt.np(bf16)
    x = np.asarray(x, dtype=np.float32)
    de_mod = np.asarray(de_mod, dtype=np.float32)
    Wb = np.asarray(Wb, dtype=np.float32)
    lora_up = np.asarray(lora_up, dtype=np.float32)
    lora_down = np.asarray(lora_down, dtype=np.float32)
    bias = np.asarray(bias, dtype=np.float32).reshape(C)

    # zero-pad x with a 1-px border; bf16
    xp = np.zeros((B, C, HP, WP), dtype=BF)
    xp[:, :, 1 : 1 + H, 1 : 1 + W] = x.astype(BF)

    # [O,I,3,3] -> [i, (t o)];  [R,C,3,3] -> [r, (t i)]
    wbT = np.ascontiguousarray(Wb.transpose(1, 2, 3, 0).reshape(C, C9))
    ld = lora_down.transpose(0, 2, 3, 1).reshape(RANK, C9)
    lu = SCALING * lora_up.T                    # [r, o], lora scale folded in
    lor = np.concatenate([ld, lu], axis=1)

    in_maps = []
    for b in range(NCORES):
        wpk = np.zeros((C, C9 + 2 + 10 * C), dtype=np.float32)
        wpk[:, 0:C9] = wbT
        wpk[:, C9] = de_mod[b]
        wpk[:, C9 + 1] = bias
        wpk[0:RANK, C9 + 2 :] = lor
        in_maps.append(
            {
                "x": np.ascontiguousarray(xp[b]),
                "wpk": wpk.astype(BF),
            }
        )
    return in_maps


def run(inputs, trace=False, trace_kwargs=None):
    nc = _get_nc()
    in_maps = _make_in_maps(**inputs)
    res = run_bass_kernel_spmd(
        nc,
        in_maps,
        core_ids=list(range(NCORES)),
        trace=trace,
        **(trace_kwargs or {}),
    )
    y = np.stack([res.results[b]["y"] for b in range(NCORES)], axis=0)
    return y, res


def kernel(**inputs):
    y, _ = run(inputs)
    return y
